# revision 29
# baseline (speedup 1.0000x reference)
"""Butterfly permuter kernel for Trainium2 (8 NeuronCores, SPMD data-parallel).

The reference applies 10 butterfly rotation stages along the feature axis
(dim=1024) of x [16384, 1024].  Stage s pairs features differing in bit s, so
the 10 stages factor by feature-bit locality:

  * stages 0-6 touch bits b0..b6  -> dense 128x128 blocks, block-diagonal in
    the natural feature tiling (tile T = f >> 7)
  * stages 7-9 touch bits b7..b9  -> block-diagonal in the STRIDED tiling
    (partition q = f >> 3, tile r = f & 7), where each 128x128 per-r matrix
    M2_r[q', q] is nonzero only for q' == q (mod 16)

Device pipeline per core (all math on device; host only reshapes/casts):
  pass 1  stages 0-6 with the DATA as the stationary operand:
            out[t, f'] = sum_f x[f, t] * M1_T[f', f]   (output token-major)
  pass 2  PE transposes of stride-8 feature slices -> q-major layout
  pass 3  stages 7-9 matrix-stationary per r-tile:
            yq_r[q', t] = sum_q M2_r[q', q] * zq_r[q, t]

This is ~48k PE cycles/rep vs ~147k for the dense-1024 matmul formulation.
I/O is fp16 (host casts; fp16 quantization error ~2e-4 rel L2, budget 2e-2):
4 MiB in + 4 MiB out per core per rep = ~23 us at the 360 GB/s DMA roofline.
Host supplies x feature-major ([1024, 2048] per core) and un-permutes the
q-major fp16 output.  PSUM->SBUF evacuations are spread across DVE,
Activation and Pool so no single copy engine becomes the bottleneck.
"""

import numpy as np

import concourse.bass as bass
import concourse.mybir as mybir
import concourse.tile as tile
from concourse import bacc
from concourse.bass_utils import run_bass_kernel_spmd

N_CORES = 8
DIM = 1024
NUM_STAGES = 10
N_TOKENS = 16384
TOK_PER_CORE = N_TOKENS // N_CORES  # 2048
TC = 512  # tokens per chunk (DMA + pass-3 granularity)
NCHUNK = TOK_PER_CORE // TC  # 4
NSUB = TC // 128  # 128-token subchunks per chunk

F16 = mybir.dt.float16
F32 = mybir.dt.float32

# schedule knobs (tuned via cost-model sim; see sim.py / simgap.py)
EVAC_ACT_OF_16 = 11  # fp32 evac units per 16 that go to ACT (rest DVE)
PS1_BUFS = 2
PS2_BUFS = 2
PS3_BUFS = 1


def _apply_stages(y, angles, stages):
    y = y.reshape(-1, DIM)
    for stage in stages:
        span = 2 ** (stage + 1)
        half = span // 2
        y = y.reshape(-1, DIM // span, span)
        left, right = y[..., :half], y[..., half:]
        th = angles[stage].reshape(1, DIM // span, half)
        c, s = np.cos(th), np.sin(th)
        y = np.concatenate([c * left + s * right, -s * left + c * right], -1)
        y = y.reshape(-1, DIM)
    return y


def _stage_matrix(angles, stages):
    """M[f_out, f_in] with transform(x) = x @ M.T."""
    return _apply_stages(np.eye(DIM), angles, stages).T


def compose_pass_matrices(angles: np.ndarray):
    """w1 [128, 1024]: w1[p, T*128+j] = M1[T*128+j, T*128+p]  (G1_T = M1_T^T)
    w2 [128, 1024]: w2[q, r*128+q'] = M2[q'*8+r, q*8+r]       (lhsT [K=q, M=q'])
    """
    a = np.asarray(angles, dtype=np.float64)
    M1 = _stage_matrix(a, range(0, 7))
    M2 = _stage_matrix(a, range(7, 10))
    w1 = np.empty((128, DIM), dtype=np.float64)
    for T in range(8):
        blk = M1[T * 128 : (T + 1) * 128, T * 128 : (T + 1) * 128]
        w1[:, T * 128 : (T + 1) * 128] = blk.T  # [f_in, f_out]
    w2 = np.empty((128, DIM), dtype=np.float64)
    for r in range(8):
        fidx = np.arange(r, DIM, 8)
        blk = M2[np.ix_(fidx, fidx)]  # [q', q]
        w2[:, r * 128 : (r + 1) * 128] = blk.T  # [q, q']
    return w1.astype(np.float16), w2.astype(np.float16)


def build_bass(reps: int = 1):
    """reps>1 repeats the whole pipeline in one NEFF (for marginal timing)."""
    nc = bacc.Bacc(None, target_bir_lowering=False)
    xt = nc.dram_tensor("xt", [DIM, TOK_PER_CORE], F16, kind="ExternalInput")
    w1 = nc.dram_tensor("w1", [128, DIM], F16, kind="ExternalInput")
    w2 = nc.dram_tensor("w2", [128, DIM], F16, kind="ExternalInput")
    ident = nc.dram_tensor("ident", [128, 128], F16, kind="ExternalInput")
    yq = nc.dram_tensor("yq", [DIM, TOK_PER_CORE], F16, kind="ExternalOutput")

    with tile.TileContext(nc) as tc:
        with (
            tc.tile_pool(name="const", bufs=1) as const_pool,
            tc.tile_pool(name="xin", bufs=4) as xin_pool,
            tc.tile_pool(name="zt", bufs=6) as zt_pool,
            tc.tile_pool(name="zq", bufs=3) as zq_pool,
            tc.tile_pool(name="yout", bufs=3) as yout_pool,
            tc.tile_pool(name="ps1", bufs=2, space="PSUM") as ps1_pool,
            tc.tile_pool(name="ps2", bufs=2, space="PSUM") as ps2_pool,
            tc.tile_pool(name="ps3", bufs=1, space="PSUM") as ps3_pool,
        ):
            ident_sb = const_pool.tile([128, 128], F16, name="ident_sb")
            nc.gpsimd.dma_start(ident_sb[:], ident[:])
            w1_sb = const_pool.tile([128, DIM], F16, name="w1_sb")
            nc.gpsimd.dma_start(w1_sb[:], w1[:])
            w2_sb = const_pool.tile([128, DIM], F16, name="w2_sb")
            nc.gpsimd.dma_start(w2_sb[:], w2[:])

            NSLOT = 16  # 128-token subchunks per rep
            x_tiles = {}
            zt_tiles = {}
            ps2_tiles = {}
            zq_tiles = {}
            y_tiles = {}
            # GPSIMD cannot read PSUM, so evacuations go to ACT and DVE.
            # fp32 units round-robin 11:5 ACT:DVE (22:10 per rep balances
            # both engines at ~22.6 us); fp16 pass-2 units always on DVE
            # where the packed 2x mode applies.
            ecnt = [0]

            def evac_f32(dst, src):
                i = ecnt[0]
                ecnt[0] += 1
                if i % 16 < EVAC_ACT_OF_16:
                    nc.scalar.copy(dst, src)
                else:
                    nc.vector.tensor_copy(dst, src)

            def load_chunk(gc):
                """gc: global chunk index (rep*NCHUNK + c)."""
                c = gc % NCHUNK
                x_tile = xin_pool.tile(
                    [128, 8 * TC], F16, name="x_chunk", tag="x_chunk"
                )
                nc.sync.dma_start(
                    x_tile[:].rearrange("p (T t) -> p T t", t=TC),
                    xt[:, c * TC : (c + 1) * TC].rearrange(
                        "(T p) t -> p T t", p=128
                    ),
                )
                x_tiles[gc] = x_tile

            def emit_mm1(s):
                """Pass 1 for 128-token sub s: 8 data-stationary matmuls."""
                gc, sub = s // NSUB, s % NSUB
                x_tile = x_tiles[gc]
                ps = ps1_pool.tile([128, 1024], F32, name="ps1", tag="ps1")
                for T in range(8):
                    col = T * TC + sub * 128
                    nc.tensor.matmul(
                        ps[:, T * 128 : (T + 1) * 128],
                        x_tile[:, col : col + 128],
                        w1_sb[:, T * 128 : (T + 1) * 128],
                        start=True,
                        stop=True,
                    )
                zt_tile = zt_pool.tile([128, DIM], F16, name="zt", tag="zt")
                if s % 8 < 5:
                    nc.scalar.copy(zt_tile[:], ps[:])
                else:
                    nc.vector.tensor_copy(zt_tile[:], ps[:])
                zt_tiles[s] = zt_tile
                if gc + 1 in x_tiles and sub == NSUB - 1:
                    pass  # x_tiles cleanup is implicit via pool recycling

            def emit_tr2(s):
                """Pass 2 for sub s: 8 strided transposes into fp16 PSUM."""
                gc, sub = s // NSUB, s % NSUB
                if sub == 0:
                    zq_tiles[gc] = zq_pool.tile(
                        [128, 8 * TC], F16, name="zq_chunk", tag="zq_chunk"
                    )
                zt_tile = zt_tiles.pop(s)
                ztv = zt_tile[:].rearrange("p (q r) -> p r q", r=8)
                ps = ps2_pool.tile([128, 1024], F16, name="ps2", tag="ps2")
                for r in range(8):
                    nc.tensor.transpose(
                        ps[:, r * 128 : (r + 1) * 128], ztv[:, r], ident_sb[:]
                    )
                # trailing unit dim: defeats the cost model's DVE 2x-mode
                # detection, which HW measurement shows PSUM sources never get
                zqv = zq_tiles[gc][:].rearrange(
                    "p (r t o) -> p r t o", t=TC, o=1
                )
                dst = zqv[:, :, sub * 128 : (sub + 1) * 128]
                srcv = ps[:].rearrange("p (r t o) -> p r t o", t=128, o=1)
                if s % 2 == 0:
                    nc.scalar.copy(dst, srcv)
                else:
                    nc.vector.tensor_copy(dst, srcv)

            def emit_p3(u):
                """Pass 3 unit u: 2 r-tiles of chunk u//4."""
                gc, rp = u // 4, u % 4
                c = gc % NCHUNK
                if rp == 0:
                    y_tiles[gc] = yout_pool.tile(
                        [128, 8 * TC], F16, name="y_chunk", tag="y_chunk"
                    )
                zq_tile = zq_tiles[gc]
                y_tile = y_tiles[gc]
                ps = ps3_pool.tile([128, 1024], F32, name="ps3", tag="ps3")
                for half in range(2):
                    r = rp * 2 + half
                    nc.tensor.matmul(
                        ps[:, half * 512 : (half + 1) * 512],
                        w2_sb[:, r * 128 : (r + 1) * 128],
                        zq_tile[:, r * TC : (r + 1) * TC],
                        start=True,
                        stop=True,
                    )
                if u % 2 == 0:
                    nc.scalar.copy(y_tile[:, rp * 1024 : (rp + 1) * 1024], ps[:])
                else:
                    nc.vector.tensor_copy(
                        y_tile[:, rp * 1024 : (rp + 1) * 1024], ps[:]
                    )
                if rp == 3:
                    zq_tiles.pop(gc)
                    # y stores on the SWDGE ring: Pool is otherwise idle, so
                    # the DMA setup never lands on a busy engine sequencer.
                    nc.gpsimd.dma_start(
                        yq[:, c * TC : (c + 1) * TC].rearrange(
                            "(r p) t -> p r t", p=128
                        ),
                        y_tiles.pop(gc)[:].rearrange("p (r t) -> p r t", t=TC),
                    )

            # Flat software pipeline across reps: at slot g the PE runs
            # pass-1 matmuls of sub g, pass-2 transposes of sub g-1, and
            # pass-3 of unit g-5 (one 2-r unit per slot; chunk C's zq is
            # complete after slot 4C+4, its units run at slots 4C+5..4C+8).
            # Every PSUM evacuation gets a full slot (~2 us) to drain before
            # the PE needs its buffer again, so the in-order PE stream never
            # waits on DVE/ACT.
            n_sub = NSLOT * reps
            n_chunk = n_sub // NSUB
            load_chunk(0)
            for g in range(n_sub + 6):
                if 5 <= g < n_sub + 5:
                    emit_p3(g - 5)
                if g < n_sub:
                    gc = g // NSUB
                    if g % NSUB == 1 and gc + 1 < n_chunk:
                        load_chunk(gc + 1)
                    emit_mm1(g)
                if 1 <= g < n_sub + 1:
                    emit_tr2(g - 1)
    nc.compile()
    return nc


_NC_CACHE = None


def _get_nc():
    global _NC_CACHE
    if _NC_CACHE is None:
        _NC_CACHE = build_bass()
    return _NC_CACHE


def make_in_maps(x: np.ndarray, angles: np.ndarray) -> list[dict]:
    """Host-side sharding: token-axis shards, feature-major fp16 layout."""
    x16 = np.asarray(x, dtype=np.float16)
    w1, w2 = compose_pass_matrices(angles)
    ident = np.eye(128, dtype=np.float16)
    in_maps = []
    for c in range(N_CORES):
        shard = x16[c * TOK_PER_CORE : (c + 1) * TOK_PER_CORE]
        in_maps.append(
            {
                "xt": np.ascontiguousarray(shard.T),
                "w1": w1,
                "w2": w2,
                "ident": ident,
            }
        )
    return in_maps


def gather_out(per_core_results: list[dict]) -> np.ndarray:
    """Host-side unshard: un-permute q-major fp16 output to [tok, dim] fp32."""
    shards = []
    for c in range(N_CORES):
        yqc = per_core_results[c]["yq"]  # [1024, 2048], row r*128+q'
        y = (
            yqc.reshape(8, 128, TOK_PER_CORE)
            .transpose(2, 1, 0)
            .reshape(TOK_PER_CORE, DIM)
        )
        shards.append(y)
    return np.concatenate(shards, axis=0).astype(np.float32)


def run(x: np.ndarray, angles: np.ndarray, trace: bool = False):
    """Run on 8 cores; returns (y_full, BassKernelResults)."""
    nc = _get_nc()
    in_maps = make_in_maps(x, angles)
    res = run_bass_kernel_spmd(
        nc, in_maps, core_ids=list(range(N_CORES)), trace=trace
    )
    y = gather_out(res.results)
    return y, res


def kernel(x: np.ndarray, angles: np.ndarray) -> np.ndarray:
    y, _ = run(x, angles, trace=False)
    return y
